# revision 1
# baseline (speedup 1.0000x reference)
"""Multi-head GAT layer (entmax15 attention over fixed-degree mailbox) on 8 trn2 cores.

Strategy (per core, dst-node sharded; full inputs in, full output out):
  - Each core owns N/8 destination nodes and their DEG=16 incoming edges.
  - Per-edge source features are fetched with `dma_gather` (int16 indices).
    Since N=50k exceeds int16 range, the node table is split into two blocks
    of <32k rows; each edge is gathered from its block, the other block's
    gather fetches a dedicated all-zeros row, and the two destinations are
    merged with one add.  Gather rows are 512B: [h (128 gdt) | P (4 f32) | pad].
  - Attention logits need only per-node scalars: p[n,h] = h[n]·(fc_w[h]^T a_src[h]),
    q[n,h] = h[n]·(fc_w[h]^T a_dst[h]).  The P table is computed on PE once per
    core and written into the gather rows' P slots, so p_src rides the gather.
  - entmax15 computed exactly with a Batcher sorting network + cumsum + threshold
    (the reference's max-subtraction is shift-invariant and skipped; all logit
    inputs are pre-scaled by 0.5 to fold the z = e/2 step).
  - m[n] = sum_d alpha[n,d] * h[src[n,d]] aggregated in input space (DVE FMA),
    transposed on PE, projected per head and head-combined in PSUM.
"""

import os
import sys
import numpy as np

sys.path.insert(0, "/opt/trn_rl_repo")

import concourse.bass as bass
import concourse.bacc as bacc
import concourse.tile as tile
from concourse import mybir
from concourse.masks import make_identity
from concourse.tile_rust import add_dep_helper
import ml_dtypes

F32 = mybir.dt.float32
I16 = mybir.dt.int16
ALU = mybir.AluOpType


# --- patch: make Tile's DMASW lane assignment respect SWDGE queue_num.
# Each SWDGE queue locks the semaphores it updates, so a Pool DMA on queue q
# must use a DMASW lane dedicated to q (8 lanes / 4 queues -> 2 lanes each).
import concourse.tile_sem_assignment as _tsa


def _patched_assign_tick(self, inst):
    import concourse.bass_isa as bass_isa_
    engine = inst.engine
    if (isinstance(inst, _tsa.DMAInst)
            and not isinstance(inst, bass_isa_.UserSyncedRemoteDMADescs)
            and engine == mybir.EngineType.Pool):
        q = getattr(inst, "queue_num", 0) or 0
        tog = getattr(self, "_gat_q_toggle", None)
        if tog is None:
            tog = self._gat_q_toggle = [0, 0, 0, 0]
        lane = q + 4 * tog[q]
        tog[q] ^= 1
        self.next_sw_dma_idx = lane
    return _tsa.TileClockTick._orig_assign_tick(self, inst)


if not hasattr(_tsa.TileClockTick, "_orig_assign_tick"):
    _tsa.TileClockTick._orig_assign_tick = _tsa.TileClockTick._assign_tick
    _tsa.TileClockTick._assign_tick = _patched_assign_tick

# ---------------------------------------------------------------- config

N = 50000
DEG = 16
DIN = 128
DOUT = 64
H = 4
CORES = 8
ROWE_BF = 256      # gather row elems (bf16): 512B
ROWE_F32 = 256     # gather row elems (f32): 1024B


class Cfg:
    def __init__(self, BR, n_own, T, S, gdt):
        assert BR % 128 == 0 and T % S == 0
        self.BR = BR                    # rows per table block
        self.NBLK = BR - 2              # real nodes per block
        self.ZR = BR - 2                # zeros row index (in-block)
        self.n_all = 2 * BR             # total table rows
        self.n_own = n_own              # padded own nodes (T*128)
        self.T = T
        self.S = S
        self.NCH = T // S
        self.gdt = gdt
        self.nck = self.n_all // 128    # PQ chunks
        self.ROWE = ROWE_F32 if gdt == F32 else ROWE_BF
        # f32-view geometry of one gather row
        self.PSTRIDE = self.ROWE // (1 if gdt == F32 else 2)
        self.POFF = DIN // (1 if gdt == F32 else 2)


def full_cfg(gdt):
    return Cfg(BR=25088, n_own=6272, T=49, S=7, gdt=gdt)


# ---------------------------------------------------------------- sort network

def batcher_stages(n=16):
    stages = []
    p = 1
    while p < n:
        k = p
        while k >= 1:
            stage = []
            for j in range(k % p, n - k, 2 * k):
                for i in range(min(k, n - j - k)):
                    if (i + j) // (p * 2) == (i + j + k) // (p * 2):
                        stage.append((i + j, i + j + k))
            stages.append((k, stage))
            k //= 2
        p *= 2
    return stages


def group_lo(los):
    los = sorted(los)
    n = len(los)
    if n == 1:
        return los[0], [[1, 1], [1, 1]]
    d = [los[i + 1] - los[i] for i in range(n - 1)]
    r = 1
    while r < n and d[r - 1] == d[0]:
        r += 1
    istride = d[0]
    if r == n:
        return los[0], [[istride * n, 1], [istride, n]]
    assert n % r == 0, (los,)
    ostride = los[r] - los[0]
    for b in range(n // r):
        for i in range(r):
            assert los[b * r + i] == los[0] + b * ostride + i * istride, (los,)
    return los[0], [[ostride, n // r], [istride, r]]


SORT_STAGES = [(k, group_lo([lo for lo, _ in st])) for (k, st) in batcher_stages(16)]


# ---------------------------------------------------------------- AP helper

def sub_ap(base_ap, off, dims):
    """AP over same tensor: keep partition dim, replace free dims (element units)."""
    return bass.AP(
        tensor=base_ap.tensor,
        offset=base_ap.offset + off,
        ap=[list(base_ap.ap[0])] + [list(d) for d in dims],
    )


# ---------------------------------------------------------------- program

def build_program(cfg, num_devices=CORES):
    nc = bacc.Bacc("TRN2", target_bir_lowering=False, debug=False,
                   num_devices=num_devices,
                   dynamic_dma_scratch_size=int(os.environ.get("GAT_DMA_SCRATCH", 65536)),
                   num_swdge_queues=4)
    gdt = cfg.gdt
    T, S, NCH = cfg.T, cfg.S, cfg.NCH
    nck = cfg.nck
    ROWE = cfg.ROWE

    # ---- DRAM tensors
    h_tab = nc.dram_tensor("h_tab", [cfg.n_all, ROWE], gdt, kind="ExternalInput").ap()
    h_cols = nc.dram_tensor("h_cols", [DIN, cfg.n_all], gdt, kind="ExternalInput").ap()
    h_cols_own = nc.dram_tensor("h_cols_own", [DIN, cfg.n_own], gdt, kind="ExternalInput").ap()
    fc_w = nc.dram_tensor("fc_w", [H, DOUT, DIN], F32, kind="ExternalInput").ap()
    attn_wT = nc.dram_tensor("attn_wT", [DOUT, 2 * H], F32, kind="ExternalInput").ap()
    fc_wT = nc.dram_tensor("fc_wT", [H, DIN, DOUT], gdt, kind="ExternalInput").ap()
    idxA_d = nc.dram_tensor("idxA", [128, T * 128], I16, kind="ExternalInput").ap()
    idxB_d = nc.dram_tensor("idxB", [128, T * 128], I16, kind="ExternalInput").ap()
    wq_d = nc.dram_tensor("wq", [128, T * DEG], F32, kind="ExternalInput").ap()
    ws64_d = nc.dram_tensor("ws64", [128, 64], F32, kind="ExternalInput").ap()
    kinv64_d = nc.dram_tensor("kinv64", [128, 64], F32, kind="ExternalInput").ap()
    out_d = nc.dram_tensor("out", [cfg.n_own, DOUT], F32, kind="ExternalOutput").ap()

    from contextlib import ExitStack
    with tile.TileContext(nc) as tc, ExitStack() as ctx:
        singles = ctx.enter_context(tc.tile_pool(name="singles", bufs=1))

        wq_sb = singles.tile([128, T * DEG], F32)
        kinv_sb = singles.tile([128, 64], F32)
        ws_sb = singles.tile([128, 64], F32)
        q_sb = singles.tile([128, T * 4], F32)
        fcwT_sb = singles.tile([128, H * DOUT], gdt)
        ident = singles.tile([128, 128], gdt)
        zero128 = singles.tile([128, DIN], gdt)
        kinvrep = singles.tile([128, S * 64], F32)
        wsrep = singles.tile([128, S * 64], F32)

        nc.sync.dma_start(out=wq_sb[:], in_=wq_d)
        nc.sync.dma_start(out=kinv_sb[:], in_=kinv64_d)
        nc.sync.dma_start(out=ws_sb[:], in_=ws64_d)
        for hh in range(H):
            nc.sync.dma_start(out=fcwT_sb[:, hh * DOUT:(hh + 1) * DOUT], in_=fc_wT[hh])
        make_identity(nc, ident[:])
        nc.vector.memset(zero128[:], 0.0)

        for dst, src in ((kinvrep, kinv_sb), (wsrep, ws_sb)):
            nc.scalar.copy(out=dst[:, 0:64], in_=src[:])
            cur = 1
            while cur < S:
                w = min(cur, S - cur)
                nc.scalar.copy(out=dst[:, cur * 64:(cur + w) * 64], in_=dst[:, 0:w * 64])
                cur += w

        # ---- stage A: U build + P table (into h_tab rows) + own q
        with tc.tile_pool(name="slab", bufs=2) as slab_pool, \
             tc.tile_pool(name="stA", bufs=1) as stA_pool, \
             tc.tile_pool(name="pqp", bufs=2, space="PSUM") as pq_pool:
            P_sb = stA_pool.tile([128, nck * 4], F32)
            U_sb = stA_pool.tile([128, 8], gdt)
            fcw_sb = stA_pool.tile([DOUT, H * DIN], F32)
            attn_sb = stA_pool.tile([DOUT, 2 * H], F32)
            for hh in range(H):
                nc.sync.dma_start(out=fcw_sb[:, hh * DIN:(hh + 1) * DIN], in_=fc_w[hh])
            nc.sync.dma_start(out=attn_sb[:], in_=attn_wT)
            u_psum = pq_pool.tile([128, 32], F32, tag="pq")
            for j in range(2 * H):
                nc.tensor.matmul(out=u_psum[:, j:j + 1],
                                 lhsT=fcw_sb[:, (j % H) * DIN:((j % H) + 1) * DIN],
                                 rhs=attn_sb[:, j:j + 1], start=True, stop=True)
            nc.scalar.copy(out=U_sb[:], in_=u_psum[:, 0:8])

            def pq_pass(cols_ap, n_chunks, rhs_ap, dst_sb_ap):
                for s0 in range(0, n_chunks, 16):
                    ns = min(16, n_chunks - s0)
                    slab = slab_pool.tile([128, 16 * 128], gdt, tag="slab")
                    nc.sync.dma_start(out=slab[:, :ns * 128],
                                      in_=cols_ap[:, s0 * 128:(s0 + ns) * 128])
                    for g in range(0, ns, 8):
                        gn = min(8, ns - g)
                        pq = pq_pool.tile([128, 32], F32, tag="pq")
                        for c in range(gn):
                            nc.tensor.matmul(out=pq[:, c * 4:(c + 1) * 4],
                                             lhsT=slab[:, (g + c) * 128:(g + c + 1) * 128],
                                             rhs=rhs_ap, start=True, stop=True)
                        nc.scalar.copy(
                            out=sub_ap(dst_sb_ap, (s0 + g) * 4, [[1, gn * 4]]),
                            in_=pq[:, :gn * 4])

            pq_pass(h_cols, nck, U_sb[:, 0:4], P_sb[:])
            pq_pass(h_cols_own, T, U_sb[:, 4:8], q_sb[:])

            # write P into the gather rows' P slots (f32 view of h_tab)
            htab_f32 = h_tab.bitcast(F32)
            if os.environ.get("GAT_NOPWRITE") == "1":
                p_write = None
            else:
                p_write = nc.sync.dma_start(
                    out=bass.AP(tensor=htab_f32.tensor, offset=cfg.POFF,
                                ap=[[cfg.PSTRIDE, 128], [128 * cfg.PSTRIDE, nck], [1, 4]]),
                    in_=P_sb[:])

        # ---- stage B
        ga_pool = ctx.enter_context(tc.tile_pool(name="ga", bufs=(S + 3) if gdt != F32 else S + 1))
        gb_pool = ctx.enter_context(tc.tile_pool(name="gb", bufs=4 if gdt != F32 else 2))
        ck_pool = ctx.enter_context(tc.tile_pool(name="ck", bufs=2))
        sc_pool = ctx.enter_context(tc.tile_pool(name="cks", bufs=1))
        m_pool = ctx.enter_context(tc.tile_pool(name="m", bufs=8))
        a64_pool = ctx.enter_context(tc.tile_pool(name="a64", bufs=4))
        mt_pool = ctx.enter_context(tc.tile_pool(name="mt", bufs=8))
        ob_pool = ctx.enter_context(tc.tile_pool(name="ob", bufs=3))
        tr_pool = ctx.enter_context(tc.tile_pool(name="tr", bufs=4, space="PSUM"))
        pr_pool = ctx.enter_context(tc.tile_pool(name="pr", bufs=2, space="PSUM"))

        idx_pool = ctx.enter_context(tc.tile_pool(name="idx", bufs=2))
        W = S * 64

        for sc in range(NCH):
            idxA_sb = idx_pool.tile([128, S * 128], I16, tag="ixa")
            idxB_sb = idx_pool.tile([128, S * 128], I16, tag="ixb")
            nc.sync.dma_start(out=idxA_sb[:],
                              in_=idxA_d[:, sc * S * 128:(sc + 1) * S * 128])
            nc.sync.dma_start(out=idxB_sb[:],
                              in_=idxB_d[:, sc * S * 128:(sc + 1) * S * 128])
            gas = []
            pt = ck_pool.tile([128, W], F32, tag="pt")
            for tl in range(S):
                t_glob = sc * S + tl
                ga = ga_pool.tile([128, DEG * ROWE], gdt, tag="ga")
                gb = gb_pool.tile([128, DEG * ROWE], gdt, tag="gb")
                for qi, (dst, isb, blk) in enumerate(((ga, idxA_sb, 0), (gb, idxB_sb, 1))):
                    d_ap = dst[:]
                    o3 = bass.AP(tensor=d_ap.tensor, offset=d_ap.offset,
                                 ap=[list(d_ap.ap[0]), [ROWE, DEG], [1, ROWE]])
                    g = nc.gpsimd.dma_gather(
                        out_ap=o3,
                        in_ap=h_tab[blk * cfg.BR:(blk + 1) * cfg.BR, :],
                        idxs_ap=isb[:, tl * 128:(tl + 1) * 128],
                        num_idxs=DEG * 128, num_idxs_reg=DEG * 128,
                        elem_size=ROWE, single_packet=False,
                        queue_num=(2 * tl + qi) % 4)
                    if p_write is not None and os.environ.get("GAT_NODEP") != "1":
                        add_dep_helper(g.ins, p_write.ins,
                                       reason="gather rows after P slot fill")
                # merge h parts: ga += gb
                nc.vector.tensor_add(
                    out=sub_ap(ga[:], 0, [[ROWE, DEG], [1, DIN]]),
                    in0=sub_ap(ga[:], 0, [[ROWE, DEG], [1, DIN]]),
                    in1=sub_ap(gb[:], 0, [[ROWE, DEG], [1, DIN]]))
                # merge + extract P parts into logits layout
                gaf = ga[:].bitcast(F32)
                gbf = gb[:].bitcast(F32)
                nc.vector.tensor_add(
                    out=sub_ap(pt[:], tl * 64, [[4, DEG], [1, 4]]),
                    in0=sub_ap(gaf, cfg.POFF, [[cfg.PSTRIDE, DEG], [1, 4]]),
                    in1=sub_ap(gbf, cfg.POFF, [[cfg.PSTRIDE, DEG], [1, 4]]))
                gas.append(ga)
            if os.environ.get("GAT_STAGE") == "gather":
                o_dbg = ob_pool.tile([128, DOUT], F32, tag="ob")
                nc.vector.tensor_copy(out=o_dbg[:], in_=pt[:, 0:DOUT])
                nc.sync.dma_start(out=out_d[sc * 128:(sc + 1) * 128, :], in_=o_dbg[:])
                continue

            z = ck_pool.tile([128, W], F32, tag="z")
            zs = sc_pool.tile([128, W], F32, tag="zs")
            A = sc_pool.tile([128, W], F32, tag="A")
            B = sc_pool.tile([128, W], F32, tag="B")
            C = sc_pool.tile([128, W], F32, tag="C")
            D = sc_pool.tile([128, W], F32, tag="D")
            E = sc_pool.tile([128, W], F32, tag="E")
            ts4 = sc_pool.tile([128, S * 4], F32, tag="ts4")

            # q replicated over d (A as scratch)
            qrep = A
            nc.scalar.copy(out=sub_ap(qrep[:], 0, [[64, S], [1, 4]]),
                           in_=sub_ap(q_sb[:], sc * S * 4, [[4, S], [1, 4]]))
            w_ = 1
            while w_ < DEG:
                ww = min(w_, DEG - w_)
                nc.scalar.copy(out=sub_ap(qrep[:], w_ * 4, [[64, S], [4, ww], [1, 4]]),
                               in_=sub_ap(qrep[:], 0, [[64, S], [4, ww], [1, 4]]))
                w_ += ww
            # wq replicated over h (B as scratch)
            wrep = B
            nc.scalar.copy(out=sub_ap(wrep[:], 0, [[64, S], [4, DEG]]),
                           in_=sub_ap(wq_sb[:], sc * S * DEG, [[16, S], [1, DEG]]))
            nc.scalar.copy(out=sub_ap(wrep[:], 1, [[64, S], [4, DEG], [1, 1]]),
                           in_=sub_ap(wrep[:], 0, [[64, S], [4, DEG], [1, 1]]))
            nc.scalar.copy(out=sub_ap(wrep[:], 2, [[64, S], [4, DEG], [1, 2]]),
                           in_=sub_ap(wrep[:], 0, [[64, S], [4, DEG], [1, 2]]))

            # logits z = lrelu(p+q) + w   (all pre-scaled by 0.5)
            nc.vector.tensor_add(out=z[:], in0=pt[:], in1=qrep[:])
            nc.vector.scalar_tensor_tensor(out=z[:], in0=z[:], scalar=0.01,
                                           in1=z[:], op0=ALU.mult, op1=ALU.max)
            nc.vector.tensor_add(out=z[:], in0=z[:], in1=wrep[:])
            if os.environ.get("GAT_STAGE") == "logits":
                o_dbg = ob_pool.tile([128, DOUT], F32, tag="ob")
                nc.vector.tensor_copy(out=o_dbg[:], in_=z[:, 0:DOUT])
                nc.sync.dma_start(out=out_d[sc * 128:(sc + 1) * 128, :], in_=o_dbg[:])
                continue

            # sort (descending) into zs; C is CE scratch
            nc.scalar.copy(out=zs[:], in_=z[:])
            for k, (lo0, dims) in SORT_STAGES:
                ap_dims = [[64, S]] + [[d[0] * 4, d[1]] for d in dims] + [[1, 4]]
                a_ap = sub_ap(zs[:], lo0 * 4, ap_dims)
                b_ap = sub_ap(zs[:], (lo0 + k) * 4, ap_dims)
                t_ap = sub_ap(C[:], lo0 * 4, ap_dims)
                nc.vector.tensor_tensor(out=t_ap, in0=a_ap, in1=b_ap, op=ALU.min)
                nc.vector.tensor_tensor(out=a_ap, in0=a_ap, in1=b_ap, op=ALU.max)
                nc.vector.tensor_copy(out=b_ap, in_=t_ap)

            def cumsum(src_t, buf1, buf2):
                bufs = [buf1, buf2]
                src = src_t
                shift = 1
                for step in range(4):
                    dst = bufs[step % 2]
                    nc.scalar.copy(out=sub_ap(dst[:], 0, [[64, S], [4, shift], [1, 4]]),
                                   in_=sub_ap(src[:], 0, [[64, S], [4, shift], [1, 4]]))
                    cnt = DEG - shift
                    nc.vector.tensor_add(
                        out=sub_ap(dst[:], shift * 4, [[64, S], [4, cnt], [1, 4]]),
                        in0=sub_ap(src[:], shift * 4, [[64, S], [4, cnt], [1, 4]]),
                        in1=sub_ap(src[:], 0, [[64, S], [4, cnt], [1, 4]]))
                    src = dst
                    shift *= 2
                return src

            if os.environ.get("GAT_STAGE") == "sort":
                o_dbg = ob_pool.tile([128, DOUT], F32, tag="ob")
                nc.vector.tensor_copy(out=o_dbg[:], in_=zs[:, 0:DOUT])
                nc.sync.dma_start(out=out_d[sc * 128:(sc + 1) * 128, :], in_=o_dbg[:])
                continue
            cs = cumsum(zs, A, B)            # ends in B
            nc.vector.tensor_mul(out=C[:], in0=zs[:], in1=zs[:])
            csq = cumsum(C, D, E)            # ends in E

            sA, sC, sD = (A, C, D)
            nc.vector.tensor_mul(out=sA[:], in0=cs[:], in1=cs[:])
            nc.vector.tensor_mul(out=sA[:], in0=sA[:], in1=kinvrep[:])
            nc.vector.tensor_sub(out=sA[:], in0=csq[:], in1=sA[:])      # ss
            nc.vector.tensor_mul(out=sD[:], in0=sA[:], in1=kinvrep[:])
            nc.vector.tensor_sub(out=sD[:], in0=kinvrep[:], in1=sD[:])  # (1-ss)/k
            nc.vector.tensor_scalar_max(out=sD[:], in0=sD[:], scalar1=0.0)
            nc.scalar.sqrt(out=sD[:], in_=sD[:])
            nc.vector.tensor_mul(out=sA[:], in0=cs[:], in1=kinvrep[:])  # mean
            nc.vector.tensor_sub(out=sA[:], in0=sA[:], in1=sD[:])      # tau

            nc.vector.tensor_tensor(out=sC[:], in0=sA[:], in1=zs[:], op=ALU.is_le)
            nc.vector.tensor_sub(
                out=sub_ap(sD[:], 0, [[64, S], [4, DEG - 1], [1, 4]]),
                in0=sub_ap(sC[:], 0, [[64, S], [4, DEG - 1], [1, 4]]),
                in1=sub_ap(sC[:], 4, [[64, S], [4, DEG - 1], [1, 4]]))
            nc.scalar.copy(out=sub_ap(sD[:], (DEG - 1) * 4, [[64, S], [1, 4]]),
                           in_=sub_ap(sC[:], (DEG - 1) * 4, [[64, S], [1, 4]]))
            nc.vector.tensor_mul(out=sD[:], in0=sD[:], in1=sA[:])
            nc.vector.tensor_reduce(
                out=ts4[:], in_=sub_ap(sD[:], 0, [[64, S], [1, 4], [4, DEG]]),
                axis=mybir.AxisListType.X, op=ALU.add)

            tsr = sC
            nc.scalar.copy(out=sub_ap(tsr[:], 0, [[64, S], [1, 4]]), in_=ts4[:])
            w_ = 1
            while w_ < DEG:
                ww = min(w_, DEG - w_)
                nc.scalar.copy(out=sub_ap(tsr[:], w_ * 4, [[64, S], [4, ww], [1, 4]]),
                               in_=sub_ap(tsr[:], 0, [[64, S], [4, ww], [1, 4]]))
                w_ += ww
            nc.vector.tensor_sub(out=z[:], in0=z[:], in1=tsr[:])
            nc.vector.tensor_scalar_max(out=z[:], in0=z[:], scalar1=0.0)
            nc.vector.tensor_mul(out=z[:], in0=z[:], in1=z[:])
            nc.vector.tensor_mul(out=z[:], in0=z[:], in1=wsrep[:])
            if os.environ.get("GAT_STAGE") == "entmax":
                o_dbg = ob_pool.tile([128, DOUT], F32, tag="ob")
                nc.vector.tensor_copy(out=o_dbg[:], in_=z[:, 0:DOUT])
                nc.sync.dma_start(out=out_d[sc * 128:(sc + 1) * 128, :], in_=o_dbg[:])
                continue

            # FMA aggregation + projection
            for tl in range(S):
                t_glob = sc * S + tl
                ga = gas[tl]
                a64 = a64_pool.tile([128, 64], gdt, tag="a64")
                nc.scalar.copy(out=a64[:], in_=z[:, tl * 64:(tl + 1) * 64])
                mts = []
                for hh in range(H):
                    m = m_pool.tile([128, DIN], gdt, tag="m")
                    for d in range(DEG):
                        a_ap = a64[:, d * 4 + hh: d * 4 + hh + 1]
                        h_ap = ga[:, d * ROWE: d * ROWE + DIN]
                        nc.vector.scalar_tensor_tensor(
                            out=m[:], in0=h_ap, scalar=a_ap,
                            in1=(zero128[:] if d == 0 else m[:]),
                            op0=ALU.mult, op1=ALU.add)
                    if os.environ.get("GAT_STAGE") == "fma":
                        mts.append(m)
                        continue
                    tr = tr_pool.tile([128, 128], gdt, tag="tr")
                    nc.tensor.transpose(out=tr[:], in_=m[:], identity=ident[:])
                    mt = mt_pool.tile([128, 128], gdt, tag="mt")
                    nc.scalar.copy(out=mt[:], in_=tr[:])
                    mts.append(mt)
                if os.environ.get("GAT_STAGE") == "fma":
                    osb = ob_pool.tile([128, DOUT], F32, tag="ob")
                    nc.vector.tensor_copy(out=osb[:], in_=mts[0][:, 0:DOUT])
                    nc.sync.dma_start(out=out_d[t_glob * 128:(t_glob + 1) * 128, :],
                                      in_=osb[:])
                    continue
                proj = pr_pool.tile([128, DOUT], F32, tag="pr")
                for hh in range(H):
                    nc.tensor.matmul(out=proj[:], lhsT=mts[hh][:],
                                     rhs=fcwT_sb[:, hh * DOUT:(hh + 1) * DOUT],
                                     start=(hh == 0), stop=(hh == H - 1))
                osb = ob_pool.tile([128, DOUT], F32, tag="ob")
                nc.scalar.copy(out=osb[:], in_=proj[:])
                nc.sync.dma_start(out=out_d[t_glob * 128:(t_glob + 1) * 128, :],
                                  in_=osb[:])

    nc.compile()
    return nc


# ---------------------------------------------------------------- host prep

def softmax_np(x):
    e = np.exp(x - np.max(x))
    return e / e.sum()


def host_prep(cfg, h, src, w, fc_w, attn_w, head_weights, n_cores, n_total=N):
    gnp = np.float32 if cfg.gdt == F32 else ml_dtypes.bfloat16
    n_own_real = n_total // n_cores
    NBLK, ZR, BR = cfg.NBLK, cfg.ZR, cfg.BR
    assert 2 * NBLK >= n_total

    # gather table: [2*BR, ROWE]; row (b*BR + r) = node b*NBLK + r
    h_tab = np.zeros((cfg.n_all, cfg.ROWE), gnp)
    hq = h.astype(gnp)
    for b in range(2):
        lo = b * NBLK
        n_here = min(NBLK, max(0, n_total - lo))
        if n_here > 0:
            h_tab[b * BR: b * BR + n_here, :DIN] = hq[lo: lo + n_here]

    # h columns in TABLE-ROW order (for the PQ pass)
    h_cols = np.ascontiguousarray(h_tab[:, :DIN].T)

    fc_w32 = fc_w.astype(np.float32)
    attn_wT = np.zeros((DOUT, 2 * H), np.float32)
    for hh in range(H):
        attn_wT[:, hh] = 0.5 * attn_w[hh, :DOUT]
        attn_wT[:, H + hh] = 0.5 * attn_w[hh, DOUT:]
    fc_wT = np.ascontiguousarray(np.transpose(fc_w, (0, 2, 1)).astype(gnp))

    ws = softmax_np(head_weights.astype(np.float32))
    ws64 = np.tile(np.tile(ws, DEG)[None, :], (128, 1)).astype(np.float32)
    kinv64 = np.tile(np.repeat(1.0 / np.arange(1, DEG + 1), H)[None, :],
                     (128, 1)).astype(np.float32)

    src2d = src.reshape(n_total, DEG).astype(np.int64)
    w2d = w.reshape(n_total, DEG).astype(np.float32)

    def pack_idx(vals_2048):
        # stream position i -> idx sbuf [16k + i%16, i//16]
        pat = np.zeros((16, 128), np.int16)
        i = np.arange(2048)
        pat[i % 16, i // 16] = vals_2048
        return pat

    in_maps = []
    for c in range(n_cores):
        lo = c * n_own_real
        hi = lo + n_own_real
        own_src = np.zeros((cfg.n_own, DEG), np.int64)
        own_src[:n_own_real] = src2d[lo:hi]
        own_w = np.zeros((cfg.n_own, DEG), np.float32)
        own_w[:n_own_real] = 0.5 * w2d[lo:hi]

        blk = own_src // NBLK
        row = own_src - blk * NBLK
        idxA = np.zeros((128, cfg.T * 128), np.int16)
        idxB = np.zeros((128, cfg.T * 128), np.int16)
        for t in range(cfg.T):
            # stream i = d*128 + p ; edge = (node t*128+p, slot d)
            d_ = np.arange(2048) // 128
            p_ = np.arange(2048) % 128
            nblk_t = blk[t * 128 + p_, d_]
            nrow_t = row[t * 128 + p_, d_]
            vA = np.where(nblk_t == 0, nrow_t, ZR).astype(np.int16)
            vB = np.where(nblk_t == 1, nrow_t, ZR).astype(np.int16)
            patA = pack_idx(vA)
            patB = pack_idx(vB)
            idxA[:, t * 128:(t + 1) * 128] = np.tile(patA, (8, 1))
            idxB[:, t * 128:(t + 1) * 128] = np.tile(patB, (8, 1))

        wq = own_w.reshape(cfg.T, 128, DEG).transpose(1, 0, 2) \
            .reshape(128, cfg.T * DEG).astype(np.float32)

        # own h columns (global node order, zero-padded)
        co = np.zeros((DIN, cfg.n_own), np.float32)
        ncols = min(cfg.n_own, n_total - lo)
        co[:, :ncols] = h[lo:lo + ncols].T
        h_cols_own = np.ascontiguousarray(co.astype(gnp))

        in_maps.append({
            "h_tab": h_tab, "h_cols": h_cols, "h_cols_own": h_cols_own,
            "fc_w": fc_w32, "attn_wT": attn_wT, "fc_wT": fc_wT,
            "idxA": idxA, "idxB": idxB, "wq": wq,
            "ws64": ws64, "kinv64": kinv64,
        })
    return in_maps


# ---------------------------------------------------------------- entry point

_PROG_CACHE = {}


def kernel(h, src, w, fc_w, attn_w, head_weights):
    h = np.asarray(h, np.float32)
    src = np.asarray(src)
    w = np.asarray(w, np.float32)
    fc_w = np.asarray(fc_w, np.float32)
    attn_w = np.asarray(attn_w, np.float32)
    head_weights = np.asarray(head_weights, np.float32)

    use_bf16 = os.environ.get("GAT_DTYPE", "bf16") == "bf16"
    gdt = mybir.dt.bfloat16 if use_bf16 else F32
    cfg = full_cfg(gdt)

    key = ("full", use_bf16)
    if key not in _PROG_CACHE:
        _PROG_CACHE[key] = build_program(cfg, num_devices=CORES)
    nc = _PROG_CACHE[key]

    in_maps = host_prep(cfg, h, src, w, fc_w, attn_w, head_weights, CORES)

    from concourse.bass_utils import run_bass_kernel_spmd
    res = run_bass_kernel_spmd(nc, in_maps, core_ids=list(range(CORES)))

    n_own_real = N // CORES
    out = np.concatenate(
        [res.results[c]["out"][:n_own_real] for c in range(CORES)], axis=0)
    return out.astype(np.float32)



# revision 4
# speedup vs baseline: 1.3809x; 1.3809x over previous
"""Multi-head GAT layer (entmax15 attention over fixed-degree mailbox) on 8 trn2 cores.

Strategy (per core, dst-node sharded; full inputs in, full output out):
  - Each core owns N/8 destination nodes and their DEG=16 incoming edges.
  - Per-edge source features are fetched with ONE `dma_gather` per 128-node
    tile from a paired-row table: row r = [h_r | h_{r+25088}] (512B, bf16),
    so int16 indices (max 25087) cover all 50k nodes.  A 3-op DVE blend
    `hsel = lo*(1-s) + hi*s` (exact for s in {0,1}) picks the right half
    using host-built masks broadcast over features via stride-0 APs.
  - Attention logits: p_src[e,h] = hsel_e . u_h computed per tile on DVE
    (stride-0 broadcast multiply + X-axis reduce); u_h = 0.5*fc_w[h]^T a_src[h]
    is host-precomputed weight prep.  q[n,h] = h[n] . (fc_w[h]^T a_dst[h]) is
    computed once on PE from the own-node column table.
  - entmax15 computed exactly with a Batcher sorting network + cumsum +
    threshold (all logit inputs pre-scaled by 0.5 to fold the z = e/2 step).
  - m[n] = sum_d alpha[n,d] * h[src[n,d]] via one wide DVE multiply
    (all 4 heads at once) + 4 halving tree adds, then PE transpose,
    per-head projection and head-combine in PSUM.
"""

import os
import sys
import numpy as np

sys.path.insert(0, "/opt/trn_rl_repo")

import concourse.bass as bass
import concourse.bacc as bacc
import concourse.tile as tile
from concourse import mybir
from concourse.masks import make_identity
import ml_dtypes

F32 = mybir.dt.float32
BF16 = mybir.dt.bfloat16
I16 = mybir.dt.int16
ALU = mybir.AluOpType


# --- patch: make Tile's DMASW lane assignment respect SWDGE queue_num.
# Each SWDGE queue locks the semaphores it updates, so a Pool DMA on queue q
# must use a DMASW lane dedicated to q (8 lanes / 4 queues -> 2 lanes each).
import concourse.tile_sem_assignment as _tsa


def _patched_assign_tick(self, inst):
    import concourse.bass_isa as bass_isa_
    engine = inst.engine
    if (isinstance(inst, _tsa.DMAInst)
            and not isinstance(inst, bass_isa_.UserSyncedRemoteDMADescs)
            and engine == mybir.EngineType.Pool):
        q = getattr(inst, "queue_num", 0) or 0
        tog = getattr(self, "_gat_q_toggle", None)
        if tog is None:
            tog = self._gat_q_toggle = [0, 0, 0, 0]
        lane = q + 4 * tog[q]
        tog[q] ^= 1
        self.next_sw_dma_idx = lane
    return _tsa.TileClockTick._orig_assign_tick(self, inst)


if not hasattr(_tsa.TileClockTick, "_orig_assign_tick"):
    _tsa.TileClockTick._orig_assign_tick = _tsa.TileClockTick._assign_tick
    _tsa.TileClockTick._assign_tick = _patched_assign_tick

# ---------------------------------------------------------------- config

N = 50000
DEG = 16
DIN = 128
DOUT = 64
H = 4
CORES = 8
NROW = 25088       # paired rows: row r = [h_r | h_{r+NROW}]
ROW = 2 * DIN      # 256 bf16 elems = 512B per row


class Cfg:
    def __init__(self, BR=None, n_own=6272, T=49, S=7, gdt=BF16):
        assert T % S == 0
        self.n_own = n_own              # padded own nodes (T*128)
        self.T = T
        self.S = S
        self.NCH = T // S
        self.gdt = BF16                 # bf16 gather table only


def full_cfg(gdt=BF16):
    return Cfg(gdt=gdt)


# ---------------------------------------------------------------- sort network

def batcher_stages(n=16):
    stages = []
    p = 1
    while p < n:
        k = p
        while k >= 1:
            stage = []
            for j in range(k % p, n - k, 2 * k):
                for i in range(min(k, n - j - k)):
                    if (i + j) // (p * 2) == (i + j + k) // (p * 2):
                        stage.append((i + j, i + j + k))
            stages.append((k, stage))
            k //= 2
        p *= 2
    return stages


def group_lo(los):
    los = sorted(los)
    n = len(los)
    if n == 1:
        return los[0], [[1, 1], [1, 1]]
    d = [los[i + 1] - los[i] for i in range(n - 1)]
    r = 1
    while r < n and d[r - 1] == d[0]:
        r += 1
    istride = d[0]
    if r == n:
        return los[0], [[istride * n, 1], [istride, n]]
    assert n % r == 0, (los,)
    ostride = los[r] - los[0]
    for b in range(n // r):
        for i in range(r):
            assert los[b * r + i] == los[0] + b * ostride + i * istride, (los,)
    return los[0], [[ostride, n // r], [istride, r]]


SORT_STAGES = [(k, group_lo([lo for lo, _ in st])) for (k, st) in batcher_stages(16)]


# ---------------------------------------------------------------- AP helper

def sub_ap(base_ap, off, dims):
    """AP over same tensor: keep partition dim, replace free dims (element units)."""
    return bass.AP(
        tensor=base_ap.tensor,
        offset=base_ap.offset + off,
        ap=[list(base_ap.ap[0])] + [list(d) for d in dims],
    )


# ---------------------------------------------------------------- program

def build_program(cfg, num_devices=CORES):
    nc = bacc.Bacc("TRN2", target_bir_lowering=False, debug=False,
                   num_devices=num_devices,
                   dynamic_dma_scratch_size=int(os.environ.get("GAT_DMA_SCRATCH", 65536)),
                   num_swdge_queues=4)
    T, S, NCH = cfg.T, cfg.S, cfg.NCH

    # ---- DRAM tensors
    h_tab = nc.dram_tensor("h_tab", [NROW, ROW], BF16, kind="ExternalInput").ap()
    h_cols_own = nc.dram_tensor("h_cols_own", [DIN, cfg.n_own], BF16, kind="ExternalInput").ap()
    fc_w = nc.dram_tensor("fc_w", [H, DOUT, DIN], F32, kind="ExternalInput").ap()
    attn_dT = nc.dram_tensor("attn_dT", [DOUT, H], F32, kind="ExternalInput").ap()
    fc_wT = nc.dram_tensor("fc_wT", [H, DIN, DOUT], BF16, kind="ExternalInput").ap()
    urep_d = nc.dram_tensor("urep", [128, H * DIN], BF16, kind="ExternalInput").ap()
    idx_d = nc.dram_tensor("idxP", [128, T * 128], I16, kind="ExternalInput").ap()
    selm_d = nc.dram_tensor("selm", [128, T * DEG], BF16, kind="ExternalInput").ap()
    selnm_d = nc.dram_tensor("selnm", [128, T * DEG], BF16, kind="ExternalInput").ap()
    wq_d = nc.dram_tensor("wq", [128, T * DEG], F32, kind="ExternalInput").ap()
    ws64_d = nc.dram_tensor("ws64", [128, 64], F32, kind="ExternalInput").ap()
    kinv64_d = nc.dram_tensor("kinv64", [128, 64], F32, kind="ExternalInput").ap()
    out_d = nc.dram_tensor("out", [cfg.n_own, DOUT], F32, kind="ExternalOutput").ap()

    from contextlib import ExitStack
    with tile.TileContext(nc) as tc, ExitStack() as ctx:
        singles = ctx.enter_context(tc.tile_pool(name="singles", bufs=1))

        wq_sb = singles.tile([128, T * DEG], F32)
        kinv_sb = singles.tile([128, 64], F32)
        ws_sb = singles.tile([128, 64], F32)
        q_sb = singles.tile([128, T * 4], F32)
        fcwT_sb = singles.tile([128, H * DOUT], BF16)
        ident = singles.tile([128, 128], BF16)
        kinvrep = singles.tile([128, S * 64], F32)
        wsrep = singles.tile([128, S * 64], F32)
        urep_sb = singles.tile([128, H * DIN], BF16)
        selm_sb = singles.tile([128, T * DEG], BF16)
        selnm_sb = singles.tile([128, T * DEG], BF16)

        nc.sync.dma_start(out=wq_sb[:], in_=wq_d)
        nc.sync.dma_start(out=kinv_sb[:], in_=kinv64_d)
        nc.sync.dma_start(out=ws_sb[:], in_=ws64_d)
        nc.sync.dma_start(out=urep_sb[:], in_=urep_d)
        nc.sync.dma_start(out=selm_sb[:], in_=selm_d)
        nc.sync.dma_start(out=selnm_sb[:], in_=selnm_d)
        for hh in range(H):
            nc.sync.dma_start(out=fcwT_sb[:, hh * DOUT:(hh + 1) * DOUT], in_=fc_wT[hh])
        make_identity(nc, ident[:])

        for dst, src in ((kinvrep, kinv_sb), (wsrep, ws_sb)):
            nc.scalar.copy(out=dst[:, 0:64], in_=src[:])
            cur = 1
            while cur < S:
                w = min(cur, S - cur)
                nc.scalar.copy(out=dst[:, cur * 64:(cur + w) * 64], in_=dst[:, 0:w * 64])
                cur += w

        # ---- stage A: own-node q table via PE
        with tc.tile_pool(name="slab", bufs=2) as slab_pool, \
             tc.tile_pool(name="stA", bufs=1) as stA_pool, \
             tc.tile_pool(name="pqp", bufs=2, space="PSUM") as pq_pool:
            U_sb = stA_pool.tile([128, 4], BF16)
            fcw_sb = stA_pool.tile([DOUT, H * DIN], F32)
            attn_sb = stA_pool.tile([DOUT, H], F32)
            for hh in range(H):
                nc.sync.dma_start(out=fcw_sb[:, hh * DIN:(hh + 1) * DIN], in_=fc_w[hh])
            nc.sync.dma_start(out=attn_sb[:], in_=attn_dT)
            u_psum = pq_pool.tile([128, 32], F32, tag="pq")
            for j in range(H):
                nc.tensor.matmul(out=u_psum[:, j:j + 1],
                                 lhsT=fcw_sb[:, j * DIN:(j + 1) * DIN],
                                 rhs=attn_sb[:, j:j + 1], start=True, stop=True)
            nc.scalar.copy(out=U_sb[:], in_=u_psum[:, 0:4])

            for s0 in range(0, T, 16):
                ns = min(16, T - s0)
                slab = slab_pool.tile([128, 16 * 128], BF16, tag="slab")
                nc.sync.dma_start(out=slab[:, :ns * 128],
                                  in_=h_cols_own[:, s0 * 128:(s0 + ns) * 128])
                for g in range(0, ns, 8):
                    gn = min(8, ns - g)
                    pq = pq_pool.tile([128, 32], F32, tag="pq")
                    for c in range(gn):
                        nc.tensor.matmul(out=pq[:, c * 4:(c + 1) * 4],
                                         lhsT=slab[:, (g + c) * 128:(g + c + 1) * 128],
                                         rhs=U_sb[:], start=True, stop=True)
                    nc.scalar.copy(
                        out=sub_ap(q_sb[:], (s0 + g) * 4, [[1, gn * 4]]),
                        in_=pq[:, :gn * 4])

        # ---- stage B
        ga_pool = ctx.enter_context(tc.tile_pool(name="ga", bufs=3))
        hs_pool = ctx.enter_context(tc.tile_pool(name="hs", bufs=S + 2))
        prod_pool = ctx.enter_context(tc.tile_pool(name="prod", bufs=2))
        ck_pool = ctx.enter_context(tc.tile_pool(name="ck", bufs=2))
        sc_pool = ctx.enter_context(tc.tile_pool(name="cks", bufs=1))
        m_pool = ctx.enter_context(tc.tile_pool(name="m", bufs=2))
        a64_pool = ctx.enter_context(tc.tile_pool(name="a64", bufs=2))
        mt_pool = ctx.enter_context(tc.tile_pool(name="mt", bufs=4))
        ob_pool = ctx.enter_context(tc.tile_pool(name="ob", bufs=3))
        idx_pool = ctx.enter_context(tc.tile_pool(name="idx", bufs=2))
        tr_pool = ctx.enter_context(tc.tile_pool(name="tr", bufs=4, space="PSUM"))
        pr_pool = ctx.enter_context(tc.tile_pool(name="pr", bufs=2, space="PSUM"))

        W = S * 64

        for sc in range(NCH):
            hsels = []
            pt = ck_pool.tile([128, W], F32, tag="pt")
            idx_sb = idx_pool.tile([128, S * 128], I16, tag="idx")
            nc.sync.dma_start(out=idx_sb[:],
                              in_=idx_d[:, sc * S * 128:(sc + 1) * S * 128])
            for tl in range(S):
                t_glob = sc * S + tl
                ga = ga_pool.tile([128, DEG * ROW], BF16, tag="ga")
                o3 = bass.AP(tensor=ga[:].tensor, offset=ga[:].offset,
                             ap=[list(ga[:].ap[0]), [ROW, DEG], [1, ROW]])
                nc.gpsimd.dma_gather(
                    out_ap=o3,
                    in_ap=h_tab,
                    idxs_ap=idx_sb[:, tl * 128:(tl + 1) * 128],
                    num_idxs=DEG * 128, num_idxs_reg=DEG * 128,
                    elem_size=ROW, single_packet=False,
                    queue_num=t_glob % 4)

                # blend: hsel = lo*(1-s) + hi*s (exact for s in {0,1});
                # prod doubles as blend scratch before the p_src multiply.
                hsel = hs_pool.tile([128, DEG * DIN], BF16, tag="hs")
                prod = prod_pool.tile([128, H * DEG * DIN], BF16, tag="prod")
                lo = sub_ap(ga[:], 0, [[ROW, DEG], [1, DIN]])
                hi = sub_ap(ga[:], DIN, [[ROW, DEG], [1, DIN]])
                t3 = sub_ap(prod[:], 0, [[DIN, DEG], [1, DIN]])
                h3 = sub_ap(hsel[:], 0, [[DIN, DEG], [1, DIN]])
                selb = sub_ap(selm_sb[:], t_glob * DEG, [[1, DEG], [0, DIN]])
                selnb = sub_ap(selnm_sb[:], t_glob * DEG, [[1, DEG], [0, DIN]])
                nc.vector.tensor_mul(out=t3, in0=hi, in1=selb)
                nc.vector.tensor_mul(out=h3, in0=lo, in1=selnb)
                nc.vector.tensor_add(out=h3, in0=h3, in1=t3)

                # p_src: pt[:, tl*64 + d*4 + h] = sum_f hsel[d,f] * u[h,f]
                pr_h = sub_ap(prod[:], 0, [[DEG * DIN, H], [DIN, DEG], [1, DIN]])
                hs_b = sub_ap(hsel[:], 0, [[0, H], [DIN, DEG], [1, DIN]])
                u_b = sub_ap(urep_sb[:], 0, [[DIN, H], [0, DEG], [1, DIN]])
                nc.vector.tensor_mul(out=pr_h, in0=hs_b, in1=u_b)
                nc.vector.tensor_reduce(
                    out=sub_ap(pt[:], tl * 64, [[1, H], [4, DEG]]),
                    in_=pr_h, axis=mybir.AxisListType.X, op=ALU.add)
                hsels.append(hsel)

            z = ck_pool.tile([128, W], F32, tag="z")
            zs = sc_pool.tile([128, W], F32, tag="zs")
            A = sc_pool.tile([128, W], F32, tag="A")
            B = sc_pool.tile([128, W], F32, tag="B")
            C = sc_pool.tile([128, W], F32, tag="C")
            D = sc_pool.tile([128, W], F32, tag="D")
            E = sc_pool.tile([128, W], F32, tag="E")
            ts4 = sc_pool.tile([128, S * 4], F32, tag="ts4")

            # q replicated over d (A as scratch)
            qrep = A
            nc.scalar.copy(out=sub_ap(qrep[:], 0, [[64, S], [1, 4]]),
                           in_=sub_ap(q_sb[:], sc * S * 4, [[4, S], [1, 4]]))
            w_ = 1
            while w_ < DEG:
                ww = min(w_, DEG - w_)
                nc.scalar.copy(out=sub_ap(qrep[:], w_ * 4, [[64, S], [4, ww], [1, 4]]),
                               in_=sub_ap(qrep[:], 0, [[64, S], [4, ww], [1, 4]]))
                w_ += ww
            # wq replicated over h (B as scratch)
            wrep = B
            nc.scalar.copy(out=sub_ap(wrep[:], 0, [[64, S], [4, DEG]]),
                           in_=sub_ap(wq_sb[:], sc * S * DEG, [[16, S], [1, DEG]]))
            nc.scalar.copy(out=sub_ap(wrep[:], 1, [[64, S], [4, DEG], [1, 1]]),
                           in_=sub_ap(wrep[:], 0, [[64, S], [4, DEG], [1, 1]]))
            nc.scalar.copy(out=sub_ap(wrep[:], 2, [[64, S], [4, DEG], [1, 2]]),
                           in_=sub_ap(wrep[:], 0, [[64, S], [4, DEG], [1, 2]]))

            # logits z = lrelu(p+q) + w   (all pre-scaled by 0.5)
            nc.vector.tensor_add(out=z[:], in0=pt[:], in1=qrep[:])
            nc.vector.scalar_tensor_tensor(out=z[:], in0=z[:], scalar=0.01,
                                           in1=z[:], op0=ALU.mult, op1=ALU.max)
            nc.vector.tensor_add(out=z[:], in0=z[:], in1=wrep[:])

            # sort (descending) into zs; C is CE scratch
            nc.scalar.copy(out=zs[:], in_=z[:])
            for k, (lo0, dims) in SORT_STAGES:
                ap_dims = [[64, S]] + [[d[0] * 4, d[1]] for d in dims] + [[1, 4]]
                a_ap = sub_ap(zs[:], lo0 * 4, ap_dims)
                b_ap = sub_ap(zs[:], (lo0 + k) * 4, ap_dims)
                t_ap = sub_ap(C[:], lo0 * 4, ap_dims)
                nc.vector.tensor_tensor(out=t_ap, in0=a_ap, in1=b_ap, op=ALU.min)
                nc.vector.tensor_tensor(out=a_ap, in0=a_ap, in1=b_ap, op=ALU.max)
                nc.vector.tensor_tensor(out=b_ap, in0=t_ap, in1=t_ap, op=ALU.max)

            def cumsum(src_t, buf1, buf2):
                bufs = [buf1, buf2]
                src = src_t
                shift = 1
                for step in range(4):
                    dst = bufs[step % 2]
                    nc.scalar.copy(out=sub_ap(dst[:], 0, [[64, S], [4, shift], [1, 4]]),
                                   in_=sub_ap(src[:], 0, [[64, S], [4, shift], [1, 4]]))
                    cnt = DEG - shift
                    nc.vector.tensor_add(
                        out=sub_ap(dst[:], shift * 4, [[64, S], [4, cnt], [1, 4]]),
                        in0=sub_ap(src[:], shift * 4, [[64, S], [4, cnt], [1, 4]]),
                        in1=sub_ap(src[:], 0, [[64, S], [4, cnt], [1, 4]]))
                    src = dst
                    shift *= 2
                return src

            cs = cumsum(zs, A, B)            # ends in B
            nc.vector.tensor_mul(out=C[:], in0=zs[:], in1=zs[:])
            csq = cumsum(C, D, E)            # ends in E

            sA, sC, sD = (A, C, D)
            nc.vector.tensor_mul(out=sA[:], in0=cs[:], in1=cs[:])
            nc.vector.tensor_mul(out=sA[:], in0=sA[:], in1=kinvrep[:])
            nc.vector.tensor_sub(out=sA[:], in0=csq[:], in1=sA[:])      # ss
            nc.vector.tensor_mul(out=sD[:], in0=sA[:], in1=kinvrep[:])
            nc.vector.tensor_sub(out=sD[:], in0=kinvrep[:], in1=sD[:])  # (1-ss)/k
            nc.vector.tensor_scalar_max(out=sD[:], in0=sD[:], scalar1=0.0)
            nc.scalar.sqrt(out=sD[:], in_=sD[:])
            nc.vector.tensor_mul(out=sA[:], in0=cs[:], in1=kinvrep[:])  # mean
            nc.vector.tensor_sub(out=sA[:], in0=sA[:], in1=sD[:])      # tau

            nc.vector.tensor_tensor(out=sC[:], in0=sA[:], in1=zs[:], op=ALU.is_le)
            nc.vector.tensor_sub(
                out=sub_ap(sD[:], 0, [[64, S], [4, DEG - 1], [1, 4]]),
                in0=sub_ap(sC[:], 0, [[64, S], [4, DEG - 1], [1, 4]]),
                in1=sub_ap(sC[:], 4, [[64, S], [4, DEG - 1], [1, 4]]))
            nc.scalar.copy(out=sub_ap(sD[:], (DEG - 1) * 4, [[64, S], [1, 4]]),
                           in_=sub_ap(sC[:], (DEG - 1) * 4, [[64, S], [1, 4]]))
            nc.vector.tensor_mul(out=sD[:], in0=sD[:], in1=sA[:])
            nc.vector.tensor_reduce(
                out=ts4[:], in_=sub_ap(sD[:], 0, [[64, S], [1, 4], [4, DEG]]),
                axis=mybir.AxisListType.X, op=ALU.add)

            tsr = sC
            nc.scalar.copy(out=sub_ap(tsr[:], 0, [[64, S], [1, 4]]), in_=ts4[:])
            w_ = 1
            while w_ < DEG:
                ww = min(w_, DEG - w_)
                nc.scalar.copy(out=sub_ap(tsr[:], w_ * 4, [[64, S], [4, ww], [1, 4]]),
                               in_=sub_ap(tsr[:], 0, [[64, S], [4, ww], [1, 4]]))
                w_ += ww
            nc.vector.tensor_sub(out=z[:], in0=z[:], in1=tsr[:])
            nc.vector.tensor_scalar_max(out=z[:], in0=z[:], scalar1=0.0)
            nc.vector.tensor_mul(out=z[:], in0=z[:], in1=z[:])
            nc.vector.tensor_mul(out=z[:], in0=z[:], in1=wsrep[:])

            # FMA aggregation (wide multiply + tree reduce) + projection
            for tl in range(S):
                t_glob = sc * S + tl
                hsel = hsels[tl]
                a64 = a64_pool.tile([128, 64], BF16, tag="a64")
                nc.scalar.copy(out=a64[:], in_=z[:, tl * 64:(tl + 1) * 64])

                prod = prod_pool.tile([128, H * DEG * DIN], BF16, tag="prod")
                pr2 = sub_ap(prod[:], 0, [[DEG * DIN, H], [DIN, DEG], [1, DIN]])
                hs2 = sub_ap(hsel[:], 0, [[0, H], [DIN, DEG], [1, DIN]])
                a_b = sub_ap(a64[:], 0, [[1, H], [4, DEG], [0, DIN]])
                nc.vector.tensor_mul(out=pr2, in0=hs2, in1=a_b)
                m_all = m_pool.tile([128, H * DIN], BF16, tag="m")
                w2 = DEG // 2
                while w2 >= 1:
                    i0 = sub_ap(prod[:], 0, [[DEG * DIN, H], [DIN, w2], [1, DIN]])
                    i1 = sub_ap(prod[:], w2 * DIN, [[DEG * DIN, H], [DIN, w2], [1, DIN]])
                    if w2 > 1:
                        nc.vector.tensor_add(out=i0, in0=i0, in1=i1)
                    else:
                        nc.vector.tensor_add(
                            out=sub_ap(m_all[:], 0, [[DIN, H], [1, DIN]]),
                            in0=i0, in1=i1)
                    w2 //= 2

                mts = []
                for hh in range(H):
                    tr = tr_pool.tile([128, 128], BF16, tag="tr")
                    nc.tensor.transpose(out=tr[:], in_=m_all[:, hh * DIN:(hh + 1) * DIN],
                                        identity=ident[:])
                    mt = mt_pool.tile([128, 128], BF16, tag="mt")
                    nc.scalar.copy(out=mt[:], in_=tr[:])
                    mts.append(mt)
                proj = pr_pool.tile([128, DOUT], F32, tag="pr")
                for hh in range(H):
                    nc.tensor.matmul(out=proj[:], lhsT=mts[hh][:],
                                     rhs=fcwT_sb[:, hh * DOUT:(hh + 1) * DOUT],
                                     start=(hh == 0), stop=(hh == H - 1))
                osb = ob_pool.tile([128, DOUT], F32, tag="ob")
                nc.scalar.copy(out=osb[:], in_=proj[:])
                nc.sync.dma_start(out=out_d[t_glob * 128:(t_glob + 1) * 128, :],
                                  in_=osb[:])

    nc.compile()
    return nc


# ---------------------------------------------------------------- host prep

def softmax_np(x):
    e = np.exp(x - np.max(x))
    return e / e.sum()


def host_prep(cfg, h, src, w, fc_w, attn_w, head_weights, n_cores, n_total=N):
    n_own_real = n_total // n_cores
    T = cfg.T

    # paired gather table: row r = [h_r | h_{r+NROW}]
    h_pad = np.zeros((2 * NROW, DIN), np.float32)
    h_pad[:n_total] = h
    h_tab = np.ascontiguousarray(
        h_pad.reshape(2, NROW, DIN).transpose(1, 0, 2).reshape(NROW, ROW)
    ).astype(ml_dtypes.bfloat16)

    fc_w32 = fc_w.astype(np.float32)
    # dst attention vectors (0.5-scaled): q = h . (fc^T a_dst) / 2
    attn_dT = np.ascontiguousarray(0.5 * attn_w[:, DOUT:].T).astype(np.float32)
    # src attention vectors folded into input space: u_h = 0.5 * fc_w[h]^T a_src[h]
    u = 0.5 * np.einsum('hof,ho->hf', fc_w32, attn_w[:, :DOUT].astype(np.float32))
    urep = np.tile(u.reshape(1, H * DIN), (128, 1)).astype(ml_dtypes.bfloat16)
    fc_wT = np.ascontiguousarray(np.transpose(fc_w, (0, 2, 1))).astype(ml_dtypes.bfloat16)

    ws = softmax_np(head_weights.astype(np.float32))
    ws64 = np.tile(np.tile(ws, DEG)[None, :], (128, 1)).astype(np.float32)
    kinv64 = np.tile(np.repeat(1.0 / np.arange(1, DEG + 1), H)[None, :],
                     (128, 1)).astype(np.float32)

    src2d = src.reshape(n_total, DEG).astype(np.int64)
    w2d = w.reshape(n_total, DEG).astype(np.float32)

    i2048 = np.arange(2048)
    d_ = i2048 // 128
    p_ = i2048 % 128

    in_maps = []
    for c in range(n_cores):
        lo = c * n_own_real
        hi = lo + n_own_real
        own_src = np.zeros((cfg.n_own, DEG), np.int64)
        own_src[:n_own_real] = src2d[lo:hi]
        own_w = np.zeros((cfg.n_own, DEG), np.float32)
        own_w[:n_own_real] = 0.5 * w2d[lo:hi]

        sel = (own_src >= NROW)
        row = np.where(sel, own_src - NROW, own_src)

        idxP = np.zeros((128, T * 128), np.int16)
        for t in range(T):
            vals = row[t * 128 + p_, d_].astype(np.int16)
            pat = np.zeros((16, 128), np.int16)
            pat[i2048 % 16, i2048 // 16] = vals
            idxP[:, t * 128:(t + 1) * 128] = np.tile(pat, (8, 1))

        self_f = sel.astype(np.float32)
        selm = self_f.reshape(T, 128, DEG).transpose(1, 0, 2) \
            .reshape(128, T * DEG).astype(ml_dtypes.bfloat16)
        selnm = (1.0 - self_f).reshape(T, 128, DEG).transpose(1, 0, 2) \
            .reshape(128, T * DEG).astype(ml_dtypes.bfloat16)

        wq = own_w.reshape(T, 128, DEG).transpose(1, 0, 2) \
            .reshape(128, T * DEG).astype(np.float32)

        # own h columns (global node order, zero-padded)
        co = np.zeros((DIN, cfg.n_own), np.float32)
        ncols = min(cfg.n_own, n_total - lo)
        co[:, :ncols] = h[lo:lo + ncols].T
        h_cols_own = np.ascontiguousarray(co).astype(ml_dtypes.bfloat16)

        in_maps.append({
            "h_tab": h_tab, "h_cols_own": h_cols_own,
            "fc_w": fc_w32, "attn_dT": attn_dT, "fc_wT": fc_wT,
            "urep": urep, "idxP": idxP, "selm": selm, "selnm": selnm,
            "wq": wq, "ws64": ws64, "kinv64": kinv64,
        })
    return in_maps


# ---------------------------------------------------------------- entry point

_PROG_CACHE = {}


def kernel(h, src, w, fc_w, attn_w, head_weights):
    h = np.asarray(h, np.float32)
    src = np.asarray(src)
    w = np.asarray(w, np.float32)
    fc_w = np.asarray(fc_w, np.float32)
    attn_w = np.asarray(attn_w, np.float32)
    head_weights = np.asarray(head_weights, np.float32)

    cfg = full_cfg()

    key = ("full",)
    if key not in _PROG_CACHE:
        _PROG_CACHE[key] = build_program(cfg, num_devices=CORES)
    nc = _PROG_CACHE[key]

    in_maps = host_prep(cfg, h, src, w, fc_w, attn_w, head_weights, CORES)

    from concourse.bass_utils import run_bass_kernel_spmd
    res = run_bass_kernel_spmd(nc, in_maps, core_ids=list(range(CORES)))

    n_own_real = N // CORES
    out = np.concatenate(
        [res.results[c]["out"][:n_own_real] for c in range(CORES)], axis=0)
    return out.astype(np.float32)
